# revision 8
# baseline (speedup 1.0000x reference)
"""Multi-head self-attention (B=4, S=2048, E=1024, H=16) on 8 TRN2 NeuronCores.

v2: collective-free seq sharding. Core c = 2*b + h owns batch b and a
balanced half of its valid query rows. Each core computes K and V for ALL
valid rows of its batch locally (replicated across the pair) -- no
inter-core communication at all, so cores are fully independent and the
whole body can sit inside a hardware For_i loop for timing.

Layout per core: compacted batch rows ordered [own queries (nq_c) | pad to
NQ | partner's rows | pad to NF]. Keys = all NF columns; queries = first NQ
columns. Zero-padded key columns give score 0 -> exp = 1, but their V-hat
ones-column entry is 0 (kmask), so they contribute exactly 0 to both the
attention numerator and the softmax denominator l. No score bias/mask is
needed anywhere (exact, not approximate).

Math notes (exactness-preserving rewrites):
- K bias dropped (softmax-invariant); V bias folded into WO bias host-side.
- 1/sqrt(D) fused into the Exp activation scale; no max-subtraction
  (scores are O(1), exp cannot overflow).
- Softmax normalizer l rides as a kmask-valued ones-column in the V-hat
  stationary tiles; 1/l is applied per head via DVE reciprocal + gpsimd
  partition-broadcast + DVE multiply (no PE broadcast matmuls).

Attention: one PSUM-resident accumulation per head-pair over all NF/128
key chunks (no SBUF stash/accumulate passes). Exp for both heads of a pair
is a single Activation over a [128, 2*NQ] PSUM tile laid out so no matmul
output window crosses a PSUM bank boundary. The score tile is single-
buffered; the PE instruction stream is ordered so enough independent work
(projections for later pairs, attn@v) sits between the exp reader and the
next scores writer that the PE never waits on the Activation engine.

PSUM budget (per partition bytes, 16384 total): scores 4608 + av ring
(2*2048 + 2*256) + proj/WO ring 3*2048 = 15360.
"""

import sys

if "/opt/trn_rl_repo" not in sys.path:
    sys.path.insert(0, "/opt/trn_rl_repo")

import numpy as np
import ml_dtypes

import concourse.bass as bass
import concourse.mybir as mybir
from concourse import bacc
from concourse.tile import TileContext

BF16 = mybir.dt.bfloat16
F32 = mybir.dt.float32

B, S, E, H = 4, 2048, 1024, 16
D = E // H          # 64
N_CORES = 8
KT = E // 128       # 8 contraction tiles
JT = E // 128       # 8 feature tiles (= head pairs)
SCALE = 1.0 / 8.0   # 1/sqrt(D)

_prog_cache = {}


def _chunks(n, lim=512):
    return [(p, min(lim, n - p)) for p in range(0, n, lim)]


def _score_windows(nq):
    """Windows of the combined 2-head score tile [128, 2*nq] (head 0 at
    [0,nq), head 1 at [nq,2*nq)) such that each matmul output window stays
    inside one 2KB PSUM bank. Requires 512 <= nq <= 640."""
    assert 512 <= nq <= 640 and nq % 64 == 0
    h0 = [(0, 512)] + ([(512, nq - 512)] if nq > 512 else [])
    h1 = [(nq, 1024 - nq)] + ([(1024, 2 * nq - 1024)] if 2 * nq > 1024 else [])
    return h0, h1


def _build_program(nq, nf, sim=False, loop=1):
    nkc = nf // 128           # key chunks
    nit = -(-nq // 128)       # output row chunks (last may be partial)
    qch = _chunks(nq)
    fch = _chunks(nf)
    h0w, h1w = _score_windows(nq)

    nc = bacc.Bacc("TRN2", target_bir_lowering=False, debug=False, num_devices=N_CORES)

    xT = nc.dram_tensor("xT", [E, nf], BF16, kind="ExternalInput").ap()
    wq = nc.dram_tensor("wq", [E, E], BF16, kind="ExternalInput").ap()
    wk = nc.dram_tensor("wk", [E, E], BF16, kind="ExternalInput").ap()
    wv = nc.dram_tensor("wv", [E, E], BF16, kind="ExternalInput").ap()
    wo = nc.dram_tensor("wo", [E, E], BF16, kind="ExternalInput").ap()
    bq = nc.dram_tensor("bq", [128, JT], F32, kind="ExternalInput").ap()
    kmask = nc.dram_tensor("kmask", [128, nkc], F32, kind="ExternalInput").ap()
    outmask = nc.dram_tensor("outmask", [128, nit], F32, kind="ExternalInput").ap()
    bo_eff = nc.dram_tensor("bo_eff", [1, E], BF16, kind="ExternalInput").ap()
    out = nc.dram_tensor("out", [nq, E], BF16, kind="ExternalOutput").ap()

    with TileContext(nc) as tc:
        with tc.tile_pool(name="persist", bufs=1) as persist:
            bq_t = persist.tile([128, JT], F32)
            nc.sync.dma_start(out=bq_t, in_=bq[:, :])
            km_t = persist.tile([128, nkc], F32)
            nc.sync.dma_start(out=km_t, in_=kmask[:, :])
            om_t = persist.tile([128, nit], F32)
            nc.sync.dma_start(out=om_t, in_=outmask[:, :])
            ones_t = persist.tile([1, 128], BF16)
            nc.vector.memset(ones_t, 1.0)
            ones8 = persist.tile([128, H], F32)
            nc.vector.memset(ones8, 1.0)
            ao_sb = [persist.tile([128, nq], BF16, name=f"ao{t}") for t in range(JT)]
            qT_sb = [persist.tile([128, nq], BF16, name=f"qT{j}") for j in range(JT)]

            with (
                tc.tile_pool(name="p_x", bufs=1) as p_x,
                tc.tile_pool(name="p_kst", bufs=1) as p_kst,
                tc.tile_pool(name="p_vh", bufs=1) as p_vh,
                tc.tile_pool(name="p_w", bufs=1) as p_w,
                tc.tile_pool(name="p2s", bufs=3) as p2s,
                tc.tile_pool(name="psA", bufs=1, space="PSUM") as psA,
            ):
                xt = [p_x.tile([128, nf], BF16, name=f"xt{k}") for k in range(KT)]
                wq_sb = [p_x.tile([128, E], BF16, name=f"wq{k}") for k in range(KT)]
                wo_sb = [p_x.tile([128, E], BF16, name=f"wo{k}") for k in range(KT)]
                kstage = [p_kst.tile([128, nf], BF16, name=f"kst{j}") for j in range(JT)]
                vhat = [p_vh.tile([128, H, D + 1], BF16, name=f"vh{v}")
                        for v in range(nkc)]
                wk_sb = [p_w.tile([128, E], BF16, name=f"wk{k}") for k in range(KT)]
                wv_sb = [p_w.tile([128, E], BF16, name=f"wv{k}") for k in range(KT)]

                _loop = None
                if loop > 1:
                    _loop = tc.For_i(
                        0, loop, 1,
                        hint_engines=(
                            mybir.EngineType.PE,
                            mybir.EngineType.Activation,
                            mybir.EngineType.DVE,
                            mybir.EngineType.SP,
                        ),
                    )
                    _loop.__enter__()

                # x first on both hwdge queues (every projection contracts
                # over all of x), then wk columns 0:256 (pairs 0-1 only --
                # the critical path to the first K piece), then the rest.
                for k in range(KT):
                    eng = nc.sync if k % 2 == 0 else nc.scalar
                    eng.dma_start(out=xt[k], in_=xT[k * 128:(k + 1) * 128, :])
                for k in range(KT):
                    eng = nc.sync if k % 2 == 0 else nc.scalar
                    eng.dma_start(
                        out=wk_sb[k][:, 0:256], in_=wk[k * 128:(k + 1) * 128, 0:256]
                    )
                for k in range(KT):
                    eng = nc.sync if k % 2 == 0 else nc.scalar
                    eng.dma_start(
                        out=wk_sb[k][:, 256:E], in_=wk[k * 128:(k + 1) * 128, 256:E]
                    )
                for k in range(KT):
                    nc.sync.dma_start(out=wq_sb[k], in_=wq[k * 128:(k + 1) * 128, :])
                for k in range(KT):
                    nc.sync.dma_start(out=wv_sb[k], in_=wv[k * 128:(k + 1) * 128, :])

                # Shared PSUM ring for projections and the output projection:
                # 2 slots of one bank each. Every projection runs as
                # sequential <=512-column pieces, each accumulating over all
                # 8 contraction tiles in one slot, so at most 2 slots (piece
                # N accumulating + piece N-1 draining into SBUF) are live.
                def p_tile(sz):
                    return psA.tile([128, sz], F32, name="ps_p", tag="p", bufs=2)

                def emit_k_piece(j, ci):
                    s0, sz = fch[ci]
                    ps = p_tile(sz)
                    for k in range(KT):
                        nc.tensor.matmul(
                            ps, wk_sb[k][:, j * 128:(j + 1) * 128],
                            xt[k][:, s0:s0 + sz],
                            start=(k == 0), stop=(k == KT - 1),
                        )
                    nc.vector.tensor_copy(kstage[j][:, s0:s0 + sz], ps)

                def emit_k(j):
                    for ci in range(len(fch)):
                        emit_k_piece(j, ci)

                def emit_q_piece(j, ci):
                    s0, sz = qch[ci]
                    ps = p_tile(sz)
                    for k in range(KT):
                        nc.tensor.matmul(
                            ps, wq_sb[k][:, j * 128:(j + 1) * 128],
                            xt[k][:, s0:s0 + sz],
                            start=(k == 0), stop=(k == KT - 1),
                        )
                    nc.vector.tensor_scalar_add(
                        qT_sb[j][:, s0:s0 + sz], ps, bq_t[:, j:j + 1]
                    )

                def emit_q(j):
                    for ci in range(len(qch)):
                        emit_q_piece(j, ci)

                def emit_v(v):
                    for fc in range(2):
                        ps = p_tile(512)
                        for k in range(KT):
                            nc.tensor.matmul(
                                ps, xt[k][:, v * 128:(v + 1) * 128],
                                wv_sb[k][:, fc * 512:(fc + 1) * 512],
                                start=(k == 0), stop=(k == KT - 1),
                            )
                        nc.vector.tensor_copy(
                            vhat[v][:, 8 * fc:8 * (fc + 1), 0:D],
                            ps.rearrange("p (h d) -> p h d", h=8),
                        )
                    # ones-column = key validity (0 on padded keys)
                    nc.vector.tensor_scalar_mul(
                        vhat[v][:, :, D:D + 1].rearrange("p h one -> p (h one)"),
                        ones8, km_t[:, v:v + 1],
                    )

                def emit_scores_exp(t, jc):
                    ps_s = psA.tile([128, 2 * nq], F32, name="ps_s", tag="s", bufs=1)
                    for hh, wins in ((0, h0w), (1, h1w)):
                        prows = slice(hh * D, (hh + 1) * D)
                        for (w0, wsz) in wins:
                            q0 = w0 - hh * nq
                            nc.tensor.matmul(
                                ps_s[:, w0:w0 + wsz],
                                kstage[t][prows, jc * 128:(jc + 1) * 128],
                                qT_sb[t][prows, q0:q0 + wsz],
                                start=True, stop=True,
                                tile_position=(hh * D, 0),
                            )
                    ph = p2s.tile([128, 2 * nq], BF16, name="ph", tag="ph", bufs=4)
                    nc.scalar.activation(
                        ph, ps_s, mybir.ActivationFunctionType.Exp, scale=SCALE,
                    )
                    return ph

                tail = nq - 512  # 0..128

                def av_views(ps):
                    """ps = (av0_hh0, av0_hh1, av1 | None). Returns per
                    (hh, chunk) output views; both heads' 64-wide tails share
                    one single-bank [65, 2*tail] tile."""
                    views = {}
                    for hh in range(2):
                        views[hh, 0] = (0, 512, ps[hh])
                        if tail and ps[2] is not None:
                            views[hh, 1] = (
                                512, tail, ps[2][:, hh * tail:(hh + 1) * tail]
                            )
                    return views

                def emit_av(t, jc, ph, views, first, last):
                    for hh in range(2):
                        h = 2 * t + hh
                        for (vhh, ci), (s0, sz, dst) in views.items():
                            if vhh != hh:
                                continue
                            # ci=1: both heads' tails share one PSUM bank and
                            # start=True clears has_written bank-wide, which
                            # would wipe the other head's first chunk. The
                            # tail tile is DVE-zeroed instead and always
                            # accumulates (start=False).
                            nc.tensor.matmul(
                                dst, vhat[jc][:, h, :],
                                ph[:, hh * nq + s0:hh * nq + s0 + sz],
                                start=(first and ci == 0), stop=last,
                            )

                def finalize(t, views):
                    # per head: ao = av[0:64] * (1 / av[64]) straight into the
                    # WO stationary layout. reciprocal + multiply on DVE, the
                    # partition broadcast on the idle gpsimd queue.
                    for (hh, ci), (s0, sz, av) in views.items():
                        rcp = p2s.tile([1, sz], F32, name="rcp",
                                       tag=f"rcp{ci}", bufs=2)
                        nc.vector.reciprocal(rcp, av[D:D + 1, :])
                        brd = p2s.tile([D, sz], F32, name="brd",
                                       tag=f"brd{ci}", bufs=2)
                        nc.gpsimd.partition_broadcast(brd, rcp)
                        nc.vector.tensor_mul(
                            ao_sb[t][hh * D:(hh + 1) * D, s0:s0 + sz],
                            av[0:D, :], brd,
                        )

                LAG = 2
                pend = []
                av_state = {}

                def pop_one():
                    t, jc, ph, first, last = pend.pop(0)
                    if t not in av_state:
                        ps = [
                            psA.tile([D + 1, 512], F32, name="ps_av",
                                     tag="av0", bufs=2)
                            for _ in range(2)
                        ]
                        if tail:
                            avt = psA.tile([D + 1, 2 * tail], F32, name="ps_avt",
                                           tag="av1", bufs=1)
                            nc.vector.memset(avt, 0.0)
                            ps.append(avt)
                        else:
                            ps.append(None)
                        av_state[t] = av_views(ps)
                    emit_av(t, jc, ph, av_state[t], first, last)
                    if last:
                        finalize(t, av_state.pop(t))

                # prefix: K/Q for pairs 0 and 1 (interleaved with the x DMA
                # arrival via tile deps -- K step k only needs xt[k]).
                emit_k(0)
                emit_q(0)
                emit_k(1)
                emit_q(1)

                # attention sweep; later-pair projections dribble into the
                # PE stream: V rides pair 0 (vhat[v] done by end of slot
                # (0, v); first consumer av(0, v) pops at slot (0, v+LAG)),
                # K(t+1) occupies slots 0..4 of pair t>=1, Q(t+1) slots 5..6.
                for t in range(JT):
                    for jc in range(nkc):
                        if t == 0:
                            emit_v(jc)
                        elif t + 1 < JT:
                            if jc < len(fch):
                                emit_k_piece(t + 1, jc)
                            elif jc < len(fch) + len(qch):
                                emit_q_piece(t + 1, jc - len(fch))
                        if t == 4 and jc == 0:
                            for k in range(KT):
                                nc.sync.dma_start(
                                    out=wo_sb[k], in_=wo[k * 128:(k + 1) * 128, :]
                                )
                            bo_t = p2s.tile([1, E], BF16, name="bo_t",
                                            tag="bo", bufs=1)
                            nc.sync.dma_start(out=bo_t, in_=bo_eff[:, :])
                        pend.append((t, jc, emit_scores_exp(t, jc),
                                     jc == 0, jc == nkc - 1))
                        while len(pend) > LAG:
                            pop_one()
                while pend:
                    pop_one()

                # output projection: out = |(ao @ wo + bo) * outmask|
                for it in range(nit):
                    r0 = it * 128
                    rsz = min(128, nq - r0)
                    for fc in range(2):
                        sl = slice(fc * 512, (fc + 1) * 512)
                        ps_o = p_tile(512)
                        for k in range(KT):
                            nc.tensor.matmul(
                                ps_o[0:rsz, :], ao_sb[k][:, r0:r0 + rsz],
                                wo_sb[k][:, sl],
                                start=(k == 0), stop=False,
                            )
                        nc.tensor.matmul(
                            ps_o[0:rsz, :], ones_t[:, 0:rsz], bo_t[:, sl],
                            start=False, stop=True,
                        )
                        o_sb = p2s.tile([rsz, 512], BF16, name="o_sb",
                                        tag="o_sb", bufs=3)
                        nc.scalar.activation(
                            o_sb, ps_o[0:rsz, :], mybir.ActivationFunctionType.Abs,
                            scale=om_t[0:rsz, it:it + 1],
                        )
                        nc.scalar.dma_start(out=out[r0:r0 + rsz, sl], in_=o_sb)

                if _loop is not None:
                    _loop.__exit__(None, None, None)
    nc.compile()
    return nc


# ---------------------------------------------------------------------------
# host side
# ---------------------------------------------------------------------------

def _layout(mask):
    """Per-core row layout. Returns (per-core list of (own_idx, partner_idx),
    NQ, NF)."""
    own = []
    for b in range(B):
        rows = np.nonzero(mask[b])[0]
        nsplit = (len(rows) + 1) // 2
        own.append((rows[:nsplit], rows[nsplit:]))
    nq_max = max(max(len(a), len(b_)) for a, b_ in own)
    nq = max(512, -(-nq_max // 64) * 64)
    nf_raw = max(nq + max(len(a), len(b_)) for a, b_ in own)
    nf = -(-nf_raw // 128) * 128
    return own, nq, nf


def build_in_maps(x, mask, WQ_w, WQ_b, WK_w, WK_b, WV_w, WV_b, WO_w, WO_b):
    x = np.asarray(x, dtype=np.float32)
    mask = np.asarray(mask).astype(bool)
    WQ_w = np.asarray(WQ_w, dtype=np.float32)
    WQ_b = np.asarray(WQ_b, dtype=np.float32)
    WK_w = np.asarray(WK_w, dtype=np.float32)
    WV_w = np.asarray(WV_w, dtype=np.float32)
    WV_b = np.asarray(WV_b, dtype=np.float32)
    WO_w = np.asarray(WO_w, dtype=np.float32)
    WO_b = np.asarray(WO_b, dtype=np.float32)

    own, nq, nf = _layout(mask)
    _prog_cache["nqf"] = (nq, nf)
    nkc = nf // 128
    nit = -(-nq // 128)

    wq_t = np.ascontiguousarray(WQ_w.T).astype(ml_dtypes.bfloat16)
    wk_t = np.ascontiguousarray(WK_w.T).astype(ml_dtypes.bfloat16)
    wv_t = np.ascontiguousarray(WV_w.T).astype(ml_dtypes.bfloat16)
    wo_t = np.ascontiguousarray(WO_w.T).astype(ml_dtypes.bfloat16)
    bq_t = np.ascontiguousarray(WQ_b.reshape(JT, 128).T)  # [128, JT] f32
    bo_eff = (WO_w @ WV_b + WO_b).astype(ml_dtypes.bfloat16).reshape(1, E)

    in_maps = []
    for c in range(N_CORES):
        b, h = divmod(c, 2)
        own_idx = own[b][h]
        part_idx = own[b][1 - h]
        nv, npp = len(own_idx), len(part_idx)
        x_sh = np.zeros((nf, E), np.float32)
        x_sh[:nv] = x[b, own_idx, :]
        x_sh[nq:nq + npp] = x[b, part_idx, :]
        xT_sh = np.ascontiguousarray(x_sh.T).astype(ml_dtypes.bfloat16)
        km = np.zeros(nf, np.float32)
        km[:nv] = 1.0
        km[nq:nq + npp] = 1.0
        km_t = np.ascontiguousarray(km.reshape(nkc, 128).T)
        om = np.zeros(nit * 128, np.float32)
        om[:nv] = 1.0
        om_t = np.ascontiguousarray(om.reshape(nit, 128).T)
        in_maps.append({
            "xT": xT_sh, "wq": wq_t, "wk": wk_t, "wv": wv_t, "wo": wo_t,
            "bq": bq_t, "kmask": km_t, "outmask": om_t, "bo_eff": bo_eff,
        })
    return in_maps


def _make_executor(nq, nf, loop=1):
    """Build the Bass program once and wrap it in a cached AOT-compiled
    shard_map with C++ fast dispatch."""
    import jax
    from jax.experimental.shard_map import shard_map
    from jax.sharding import Mesh, PartitionSpec, NamedSharding
    from concourse.bass2jax import (
        _bass_exec_p,
        install_neuronx_cc_hook,
        partition_id_tensor,
    )

    nc = _build_program(nq, nf, loop=loop)
    install_neuronx_cc_hook()
    assert nc.dbg_addr is None
    partition_name = nc.partition_id_tensor.name if nc.partition_id_tensor else None

    in_names, out_names, out_avals, zero_outs = [], [], [], []
    for alloc in nc.m.functions[0].allocations:
        if not isinstance(alloc, mybir.MemoryLocationSet):
            continue
        name = alloc.memorylocations[0].name
        if alloc.kind == "ExternalInput":
            if name != partition_name:
                in_names.append(name)
        elif alloc.kind == "ExternalOutput":
            shape = tuple(alloc.tensor_shape)
            dtype = mybir.dt.np(alloc.dtype)
            out_names.append(name)
            out_avals.append(jax.core.ShapedArray(shape, dtype))
            zero_outs.append(np.zeros(shape, dtype))
    n_params = len(in_names)
    n_outs = len(out_avals)
    all_names = in_names + out_names
    if partition_name is not None:
        all_names = all_names + [partition_name]
    donate = tuple(range(n_params, n_params + n_outs))

    def _body(*args):
        operands = list(args)
        if partition_name is not None:
            operands.append(partition_id_tensor())
        outs = _bass_exec_p.bind(
            *operands,
            out_avals=tuple(out_avals),
            in_names=tuple(all_names),
            out_names=tuple(out_names),
            lowering_input_output_aliases=(),
            sim_require_finite=True,
            sim_require_nnan=True,
            nc=nc,
        )
        return tuple(outs)

    devices = jax.devices()[:N_CORES]
    mesh = Mesh(np.asarray(devices), ("core",))
    in_specs = (PartitionSpec("core"),) * (n_params + n_outs)
    out_specs = (PartitionSpec("core"),) * n_outs
    sharding = NamedSharding(mesh, PartitionSpec("core"))

    def _make_jit():
        return jax.jit(
            shard_map(_body, mesh=mesh, in_specs=in_specs, out_specs=out_specs,
                      check_rep=False),
            donate_argnums=donate,
            keep_unused=True,
        )

    try:
        from concourse.bass2jax import fast_dispatch_compile

        arg_shapes = []
        for alloc in nc.m.functions[0].allocations:
            if not isinstance(alloc, mybir.MemoryLocationSet):
                continue
            name = alloc.memorylocations[0].name
            if (alloc.kind == "ExternalInput" and name in in_names) or (
                alloc.kind == "ExternalOutput" and name in out_names
            ):
                shape = tuple(alloc.tensor_shape)
                dtype = mybir.dt.np(alloc.dtype)
                arg_shapes.append(
                    (name, jax.ShapeDtypeStruct(
                        (N_CORES * shape[0], *shape[1:]), dtype, sharding=sharding))
                )
        order = {n: i for i, n in enumerate(in_names + out_names)}
        args = [s for _, s in sorted(arg_shapes, key=lambda t: order[t[0]])]
        sharded = fast_dispatch_compile(
            lambda: _make_jit().lower(*args).compile()
        )
    except Exception:
        sharded = _make_jit()
    return {
        "jit": sharded, "in_names": in_names, "out_names": out_names,
        "out_avals": out_avals, "zero_outs": zero_outs, "sharding": sharding,
        "jax": jax,
    }


def get_executor(nqf=None, loop=1):
    if nqf is None:
        nqf = _prog_cache["nqf"]
    key = ("ex", nqf, loop)
    if key not in _prog_cache:
        _prog_cache[key] = _make_executor(nqf[0], nqf[1], loop=loop)
    return _prog_cache[key]


def run_spmd(in_maps, nqf, loop=1):
    """Execute on 8 cores; returns list of per-core output dicts (with
    tunnel-hiccup retry)."""
    import time as _time

    last_err = None
    for attempt in range(3):
        try:
            ex = get_executor(nqf, loop=loop)
            jax = ex["jax"]
            concat_in = [
                np.concatenate([np.asarray(m[name]) for m in in_maps], axis=0)
                for name in ex["in_names"]
            ]
            concat_zeros = [
                np.zeros((N_CORES * z.shape[0], *z.shape[1:]), z.dtype)
                for z in ex["zero_outs"]
            ]
            out_arrs = ex["jit"](*concat_in, *concat_zeros)
            out_arrs = [np.asarray(a) for a in out_arrs]
            return [
                {
                    name: out_arrs[i].reshape(N_CORES, *ex["out_avals"][i].shape)[c]
                    for i, name in enumerate(ex["out_names"])
                }
                for c in range(N_CORES)
            ]
        except Exception as e:
            last_err = e
            for key in [k for k in _prog_cache if isinstance(k, tuple)]:
                del _prog_cache[key]
            _time.sleep(45)
            try:
                import jax as _jax
                _jax.clear_caches()
                from jax._src import api as _japi
                _japi.clear_backends()
            except Exception:
                pass
    raise last_err


def kernel(x, mask, WQ_w, WQ_b, WK_w, WK_b, WV_w, WV_b, WO_w, WO_b):
    mask = np.asarray(mask).astype(bool)
    in_maps = build_in_maps(x, mask, WQ_w, WQ_b, WK_w, WK_b, WV_w, WV_b, WO_w, WO_b)
    own, nq, nf = _layout(mask)
    results = run_spmd(in_maps, (nq, nf))
    out = np.zeros((B, S, E), dtype=np.float32)
    for c in range(N_CORES):
        b, h = divmod(c, 2)
        own_idx = own[b][h]
        out[b, own_idx, :] = results[c]["out"][:len(own_idx)].astype(np.float32)
    return out
